# revision 1
# baseline (speedup 1.0000x reference)
"""Trainium2 Bass kernel for the DifferentiableQuantumCircuit problem.

Math: output = |U x / ||x|| |^2 with U = kron of 12 single-qubit U3 gates
applied twice (2 layers). Gates on different qubits commute, so the two
layers fuse into ONE kron-product unitary with per-qubit gates
G_q = U3_layer2(q) @ U3_layer1(q).

State index split: i = q5 * 128 + l7, with q5 = qubits 0-4 (5 MSBs) and
l7 = qubits 5-11 (7 LSBs, contiguous in memory -> 512B DMA bursts).
U_total = M5a (x) M7b with M5a = kron(G_0..G_4) [32x32] acting on q5 and
M7b = kron(G_5..G_11) [128x128] acting on l7.

Per-core pipeline (512 samples/core, 4 chunks of 128 samples b=(bh,b2),
bh in [0,32), b2 in [0,4); chunks split into 2 halves of 16 bh each):
  1. DMA-load half: Xh[(b2,q5), (bh,l7)] = x[bh*4+b2, q5*128+l7]
  2. stage 1 (PE "trick" matmuls): stationary = Xh column-chunk (fixed
     bh), moving = [Re(G5bd^T) | Im(G5bd^T)] with G5bd = I4 (x) M5a
     acting on the (b2,q5) partition index -> psum[l7, (re/im,(b2',q5'))]
     (applies the 5-qubit gate group AND transposes l7 onto partitions)
  3. evacuate psum -> S1 group tiles with 1/||x_b|| fused (broadcast-AP
     multiply on VectorE)
  4. stage 2: stationary = S1r/S1i column-chunks [l7, (b2',q5')], moving
     = [Re(M7b^T)|Im(M7b^T)] / [-Im|Re], accumulating
     -> psum[(b2',q5'), (re/im, l7')]
  5. squares on ScalarE, re^2+im^2 add on GpSimd (per half)
  6. DMA-store Ph[(b2,q5'), (bh, l7')] -> out[b, i]  (512B bursts)

Norm chain (per chunk): x^2 (ScalarE) -> 128-segment reduce (VectorE) ->
block-diag-ones matmul (PE, sums over q5 per b2 group) -> tiny DMAs to a
single-partition row -> sqrt (ScalarE) -> reciprocal (VectorE) ->
ones-column matmul broadcast to all partitions (PE) -> TRBC tile.
"""

from contextlib import ExitStack

import numpy as np

import concourse.tile as tile
from concourse import bacc, mybir
from concourse.bass_utils import run_bass_kernel_spmd

F32 = mybir.dt.float32
F32R = mybir.dt.float32r

NUM_QUBITS = 12
D = 4096
B = 4096
N_CORES = 8
B_CORE = B // N_CORES  # 512
CHUNK = 128
N_CHUNKS = B_CORE // CHUNK  # 4
GROUP = 4  # c-tiles per psum group tile (2 banks)
HALF = D // 2  # free columns per half-chunk (16 bh x 128 l7)


def _u3(theta, phi, lam):
    """Single-qubit U3 gate, complex128 [2,2] (same formula as reference)."""
    c = np.cos(theta / 2.0)
    s = np.sin(theta / 2.0)
    return np.array(
        [
            [c, -np.exp(1j * lam) * s],
            [np.exp(1j * phi) * s, np.exp(1j * (phi + lam)) * c],
        ],
        dtype=np.complex128,
    )


def _gate_consts(thetas, phis, lams):
    """Build the constant moving-operand matrices for both PE stages."""
    thetas = np.asarray(thetas, dtype=np.float64)
    phis = np.asarray(phis, dtype=np.float64)
    lams = np.asarray(lams, dtype=np.float64)
    gates = []
    for q in range(NUM_QUBITS):
        g1 = _u3(thetas[0, q], phis[0, q], lams[0, q])
        g2 = _u3(thetas[1, q], phis[1, q], lams[1, q])
        gates.append(g2 @ g1)  # layer 1 applied first, then layer 2

    m5a = gates[0]
    for q in range(1, 5):
        m5a = np.kron(m5a, gates[q])  # [32,32], acts on q5 (bits 0-4)
    m7b = gates[5]
    for q in range(6, 12):
        m7b = np.kron(m7b, gates[q])  # [128,128], acts on l7 (bits 5-11)

    g5 = np.kron(np.eye(4), m5a)  # [128,128] block-diag over (b2, q5)

    mv1 = np.concatenate([g5.T.real, g5.T.imag], axis=1)  # [128,256]
    mv2a = np.concatenate([m7b.T.real, m7b.T.imag], axis=1)
    mv2b = np.concatenate([-m7b.T.imag, m7b.T.real], axis=1)
    return (
        np.ascontiguousarray(mv1, dtype=np.float32),
        np.ascontiguousarray(mv2a, dtype=np.float32),
        np.ascontiguousarray(mv2b, dtype=np.float32),
    )


def _build_nc():
    nc = bacc.Bacc(
        "TRN2", target_bir_lowering=False, debug=False, num_devices=N_CORES
    )
    x_ap = nc.dram_tensor("x", [B_CORE, D], F32R, kind="ExternalInput").ap()
    mv1_ap = nc.dram_tensor("mv1", [128, 256], F32R, kind="ExternalInput").ap()
    mv2a_ap = nc.dram_tensor("mv2a", [128, 256], F32R, kind="ExternalInput").ap()
    mv2b_ap = nc.dram_tensor("mv2b", [128, 256], F32R, kind="ExternalInput").ap()
    out_ap = nc.dram_tensor("probs", [B_CORE, D], F32, kind="ExternalOutput").ap()

    with tile.TileContext(nc) as tc, ExitStack() as ctx:
        xpool0 = ctx.enter_context(tc.tile_pool(name="xp", bufs=8))
        all_Xh = [[None, None] for _ in range(N_CHUNKS)]

        def emit_load(k):
            xflat = x_ap[k * CHUNK : (k + 1) * CHUNK, :].flatten()
            QTR = HALF // 2
            for h in range(2):
                X = xpool0.tile([128, HALF], F32R, tag="X")
                all_Xh[k][h] = X
                for q in range(2):
                    nc.sync.dma_start(
                        X[:, q * QTR : (q + 1) * QTR].rearrange(
                            "p (bh l) -> p bh l", l=128
                        ),
                        xflat[
                            (2 * h + q) * CHUNK * QTR : (2 * h + q + 1) * CHUNK * QTR
                        ].rearrange("(bh p l) -> p bh l", p=128, l=128),
                    )

        # chunk 0, half 0 in eighth-granularity so the first stage-1 group
        # (bh 0-3) can start as early as possible
        xflat0 = x_ap[0:CHUNK, :].flatten()
        EGT = HALF // 4
        X00 = xpool0.tile([128, HALF], F32R, tag="X")
        all_Xh[0][0] = X00
        for e in range(4):
            nc.sync.dma_start(
                X00[:, e * EGT : (e + 1) * EGT].rearrange(
                    "p (bh l) -> p bh l", l=128
                ),
                xflat0[e * CHUNK * EGT : (e + 1) * CHUNK * EGT].rearrange(
                    "(bh p l) -> p bh l", p=128, l=128
                ),
            )
        QTR0 = HALF // 2
        X01 = xpool0.tile([128, HALF], F32R, tag="X")
        all_Xh[0][1] = X01
        for q in range(2):
            nc.sync.dma_start(
                X01[:, q * QTR0 : (q + 1) * QTR0].rearrange(
                    "p (bh l) -> p bh l", l=128
                ),
                xflat0[
                    (2 + q) * CHUNK * QTR0 : (3 + q) * CHUNK * QTR0
                ].rearrange("(bh p l) -> p bh l", p=128, l=128),
            )

        consts = ctx.enter_context(tc.tile_pool(name="consts", bufs=1))
        mv1_tt = consts.tile([128, 256], F32R, tag="mv1")
        nc.sync.dma_start(mv1_tt[:], mv1_ap[:])
        mv2a_tt = consts.tile([128, 256], F32R, tag="mv2a")
        nc.sync.dma_start(mv2a_tt[:], mv2a_ap[:])
        mv2b_tt = consts.tile([128, 256], F32R, tag="mv2b")
        nc.sync.dma_start(mv2b_tt[:], mv2b_ap[:])
        mv1_t = mv1_tt[:]
        mv2a_t = mv2a_tt[:]
        mv2b_t = mv2b_tt[:]
        # I4 (x) ones32: sums over q5 within each b2 block
        bdones_t = consts.tile([128, 128], F32, tag="bdones")
        nc.vector.memset(bdones_t[:], 0.0)
        for b2 in range(4):
            s = slice(b2 * 32, (b2 + 1) * 32)
            nc.vector.memset(bdones_t[s, s], 1.0)
        # single-partition ones column for the partition-broadcast matmul
        onescol_t = consts.tile([1, 128], F32, tag="onescol")
        nc.vector.memset(onescol_t[:], 1.0)

        bigp = ctx.enter_context(tc.tile_pool(name="bigp", bufs=3))
        smallp = ctx.enter_context(tc.tile_pool(name="smallp", bufs=4))
        trbcp = ctx.enter_context(tc.tile_pool(name="trbcp", bufs=8))
        s1pool = ctx.enter_context(tc.tile_pool(name="s1p", bufs=8))
        ppool = ctx.enter_context(tc.tile_pool(name="pp", bufs=4))
        ps1 = ctx.enter_context(tc.tile_pool(name="ps1", bufs=2, space="PSUM"))
        ps2 = ctx.enter_context(tc.tile_pool(name="ps2", bufs=2, space="PSUM"))

        # ---- prologue: remaining input DMAs + per-chunk norm chains ----
        def emit_sumsq(k):
            seg = smallp.tile([128, 32], F32, tag="seg")
            QTR = HALF // 2
            for h in range(2):
                for q in range(2):
                    x2 = bigp.tile([128, QTR], F32, tag="x2")
                    nc.scalar.square(
                        x2[:], all_Xh[k][h][:, q * QTR : (q + 1) * QTR].bitcast(F32)
                    )
                    qq = h * 2 + q
                    nc.vector.tensor_reduce(
                        seg[:, qq * 8 : (qq + 1) * 8],
                        x2[:].rearrange("p (bh l) -> p bh l", l=128),
                        axis=mybir.AxisListType.X,
                        op=mybir.AluOpType.add,
                    )
            return seg

        def emit_norm(seg):
            psv = ps2.tile([128, 32], F32, tag="g2")
            nc.tensor.matmul(
                psv[:], lhsT=bdones_t[:], rhs=seg[:], start=True, stop=True
            )
            psvs = smallp.tile([128, 32], F32, tag="psvs")
            nc.vector.tensor_copy(psvs[:], psv[:])
            t1s = smallp.tile([1, 128], F32, tag="t1s")
            for b2 in range(4):
                nc.sync.dma_start(
                    t1s[0:1, b2 * 32 : (b2 + 1) * 32],
                    psvs[b2 * 32 : b2 * 32 + 1, :],
                )
            t1sq = smallp.tile([1, 128], F32, tag="t1sq")
            nc.scalar.sqrt(t1sq[:], t1s[:])
            t1inv = smallp.tile([1, 128], F32, tag="t1inv")
            nc.vector.reciprocal(t1inv[:], t1sq[:])
            psb = ps2.tile([128, 128], F32, tag="g2")
            nc.tensor.matmul(
                psb[:], lhsT=onescol_t[:], rhs=t1inv[:], start=True, stop=True
            )
            trbc = trbcp.tile([128, 128], F32, tag="trbc")
            nc.vector.tensor_copy(trbc[:], psb[:])
            return trbc

        all_trbc = [None] * N_CHUNKS
        for k in range(N_CHUNKS):
            if k > 0:
                emit_load(k)
            all_trbc[k] = emit_norm(emit_sumsq(k))

        for k in range(N_CHUNKS):
            # ---- gate stages per half (norm chain emitted in phase A) ----
            for h in range(2):
                trbc = all_trbc[k]
                X = all_Xh[k][h]
                T1 = bigp.tile([128, HALF], F32, tag="T1")
                T2 = bigp.tile([128, HALF], F32, tag="T2")
                for gl in range(4):  # groups within this half
                    g = h * 4 + gl
                    # stage 1 group
                    pg = ps1.tile([128, GROUP * 256], F32, tag="g1")
                    for j in range(GROUP):
                        cl = gl * GROUP + j  # c-tile local to half
                        nc.tensor.matmul(
                            pg[:, j * 256 : (j + 1) * 256],
                            lhsT=X[:, cl * 128 : (cl + 1) * 128],
                            rhs=mv1_t,
                            start=True,
                            stop=True,
                        )
                    # evacuate with 1/||x|| scaling (varies per (j, b2'))
                    S1r = s1pool.tile([128, GROUP * 128], F32R, tag="S1r")
                    S1i = s1pool.tile([128, GROUP * 128], F32R, tag="S1i")
                    pg4 = pg[:].rearrange(
                        "p (j r b2 q) -> p j r b2 q", j=GROUP, r=2, b2=4
                    )
                    vb = (
                        trbc[:]
                        .rearrange("p (b2 bh) -> p b2 bh", b2=4)[
                            :, :, g * GROUP : (g + 1) * GROUP
                        ]
                        .transpose([0, 2, 1])
                        .unsqueeze(3)
                        .broadcast_to([128, GROUP, 4, 32])
                    )
                    nc.vector.tensor_tensor(
                        S1r[:].rearrange("p (j b2 q) -> p j b2 q", j=GROUP, b2=4),
                        pg4[:, :, 0],
                        vb,
                        op=mybir.AluOpType.mult,
                    )
                    nc.vector.tensor_tensor(
                        S1i[:].rearrange("p (j b2 q) -> p j b2 q", j=GROUP, b2=4),
                        pg4[:, :, 1],
                        vb,
                        op=mybir.AluOpType.mult,
                    )
                    # stage 2 group
                    pg2 = ps2.tile([128, GROUP * 256], F32, tag="g2")
                    for j in range(GROUP):
                        cc = slice(j * 128, (j + 1) * 128)
                        nc.tensor.matmul(
                            pg2[:, j * 256 : (j + 1) * 256],
                            lhsT=S1r[:, cc],
                            rhs=mv2a_t,
                            start=True,
                            stop=False,
                        )
                        nc.tensor.matmul(
                            pg2[:, j * 256 : (j + 1) * 256],
                            lhsT=S1i[:, cc],
                            rhs=mv2b_t,
                            start=False,
                            stop=True,
                        )
                    pg3 = pg2[:].rearrange("p (j n) -> p j n", n=256)
                    gcols = slice(gl * GROUP * 128, (gl + 1) * GROUP * 128)
                    nc.scalar.square(
                        T1[:, gcols].rearrange("p (j n) -> p j n", n=128),
                        pg3[:, :, 0:128],
                    )
                    nc.scalar.square(
                        T2[:, gcols].rearrange("p (j n) -> p j n", n=128),
                        pg3[:, :, 128:256],
                    )

                # probs = re^2 + im^2 ; store back to [b, i] (512B bursts)
                P = ppool.tile([128, HALF], F32, tag="P")
                add_eng = nc.vector if (k == N_CHUNKS - 1 and h == 1) else nc.gpsimd
                for gl in range(4):
                    gc = slice(gl * GROUP * 128, (gl + 1) * GROUP * 128)
                    add_eng.tensor_tensor(
                        P[:, gc], T1[:, gc], T2[:, gc], op=mybir.AluOpType.add
                    )
                oflat = out_ap[k * CHUNK : (k + 1) * CHUNK, :].flatten()
                QTR = HALF // 2
                nparts = 2 if (k == N_CHUNKS - 1 and h == 1) else 1
                step = HALF // nparts
                for q in range(nparts):
                    base = h * CHUNK * HALF + q * CHUNK * step
                    nc.scalar.dma_start(
                        oflat[base : base + CHUNK * step].rearrange(
                            "(bh p l) -> p bh l", p=128, l=128
                        ),
                        P[:, q * step : (q + 1) * step].rearrange(
                            "p (bh l) -> p bh l", l=128
                        ),
                    )

    nc.compile()
    return nc


_NC_CACHE = {}


def _get_nc():
    if "nc" not in _NC_CACHE:
        _NC_CACHE["nc"] = _build_nc()
    return _NC_CACHE["nc"]


def kernel(inputs, thetas, phis, lams, _trace=False, _trace_kwargs=None):
    inputs = np.ascontiguousarray(np.asarray(inputs), dtype=np.float32)
    mv1, mv2a, mv2b = _gate_consts(thetas, phis, lams)

    nc = _get_nc()
    in_maps = [
        {
            "x": inputs[k * B_CORE : (k + 1) * B_CORE],
            "mv1": mv1,
            "mv2a": mv2a,
            "mv2b": mv2b,
        }
        for k in range(N_CORES)
    ]
    res = run_bass_kernel_spmd(
        nc, in_maps, list(range(N_CORES)), trace=_trace, **(_trace_kwargs or {})
    )
    out = np.concatenate([res.results[k]["probs"] for k in range(N_CORES)], axis=0)
    if _trace:
        kernel.last_result = res
    return out



# revision 6
# speedup vs baseline: 1.1185x; 1.1185x over previous
"""Trainium2 Bass kernel for the DifferentiableQuantumCircuit problem.

Math: output = |U x / ||x|| |^2 with U = kron of 12 single-qubit U3 gates
applied twice (2 layers). Gates on different qubits commute, so the two
layers fuse into ONE kron-product unitary with per-qubit gates
G_q = U3_layer2(q) @ U3_layer1(q).

State index split: i = q5 * 128 + l7, with q5 = qubits 0-4 (5 MSBs) and
l7 = qubits 5-11 (7 LSBs, contiguous in memory -> 512B DMA bursts).
U_total = M5a (x) M7b with M5a = kron(G_0..G_4) [32x32] acting on q5 and
M7b = kron(G_5..G_11) [128x128] acting on l7.

Per-core pipeline (512 samples/core, 4 chunks of 128 samples b=(bh,b2),
bh in [0,32), b2 in [0,4)); per chunk, groups of 4 bh:
  stage 1 (PE, f32r): stationary = X c-tile (fixed bh), moving =
    [Re(G5^T)|Im(G5^T)] with G5 = I4 (x) M5a -> psum[l7, (re/im,(b2,q5))]
    (applies the 5-qubit gate group AND transposes l7 onto partitions)
  evac (V/S split): psum f32 -> SBUF bf16 S1 tiles
  stage 2 (PE, bf16): stationary = S1 re/im slices, moving =
    [Re(M7b^T)|Im(M7b^T)] / [-Im|Re] accumulating -> psum[(b2,q5'), (re/im, l7')]
  squares (ScalarE): psum f32 -> T bf16; pair add (VectorE) -> Pf f32
  norm: x^2 (ScalarE) -> per-bh l7-reduce (VectorE) -> block-diag-ones
    matmul (PE, sums over q5) -> reciprocal (VectorE) = 1/||x||^2
  final scale (GpSimdE): Pf * invnorm2 broadcast -> PfS -> DMA store

Normalization is folded at the END (probs_unnorm * 1/||x||^2), keeping the
norm chain off the stage-1/stage-2 critical path. All matmul weights are
bf16 except stage-1 (f32r X); tolerance budget is 2e-2, bf16 lands ~3e-3.
"""

from contextlib import ExitStack

import numpy as np
import ml_dtypes

import concourse.tile as tile
from concourse import bacc, mybir
from concourse.alu_op_type import AluOpType
from concourse.bass_utils import run_bass_kernel_spmd

F32 = mybir.dt.float32
F32R = mybir.dt.float32r
BF16 = mybir.dt.bfloat16

NUM_QUBITS = 12
D = 4096
B = 4096
N_CORES = 8
B_CORE = B // N_CORES  # 512
CHUNK = 128
N_CHUNKS = B_CORE // CHUNK  # 4
NG = 8  # stage groups per chunk (4 bh each)
GW = 4 * 256  # S1/psum cols per group: 4 bh x (re|im) x 128


def _u3(theta, phi, lam):
    c = np.cos(theta / 2.0)
    s = np.sin(theta / 2.0)
    return np.array(
        [
            [c, -np.exp(1j * lam) * s],
            [np.exp(1j * phi) * s, np.exp(1j * (phi + lam)) * c],
        ],
        dtype=np.complex128,
    )


def _gate_consts(thetas, phis, lams):
    """Constant moving-operand matrices for both PE stages + bdones."""
    thetas = np.asarray(thetas, dtype=np.float64)
    phis = np.asarray(phis, dtype=np.float64)
    lams = np.asarray(lams, dtype=np.float64)
    gates = []
    for q in range(NUM_QUBITS):
        g1 = _u3(thetas[0, q], phis[0, q], lams[0, q])
        g2 = _u3(thetas[1, q], phis[1, q], lams[1, q])
        gates.append(g2 @ g1)  # layer 1 applied first, then layer 2

    m5a = gates[0]
    for q in range(1, 5):
        m5a = np.kron(m5a, gates[q])  # [32,32], acts on q5 (bits 0-4)
    m7b = gates[5]
    for q in range(6, 12):
        m7b = np.kron(m7b, gates[q])  # [128,128], acts on l7 (bits 5-11)

    g5 = np.kron(np.eye(4), m5a)  # [128,128] block-diag over (b2, q5)

    mv1 = np.concatenate([g5.T.real, g5.T.imag], axis=1)  # [128,256]
    mv2a = np.concatenate([m7b.T.real, m7b.T.imag], axis=1)
    mv2b = np.concatenate([-m7b.T.imag, m7b.T.real], axis=1)
    bdones = np.kron(np.eye(4), np.ones((32, 32)))  # sums over q5 per b2
    return (
        np.ascontiguousarray(mv1, dtype=np.float32),
        np.ascontiguousarray(mv2a.astype(np.float32), dtype=np.float32).astype(
            ml_dtypes.bfloat16
        ),
        np.ascontiguousarray(mv2b.astype(np.float32), dtype=np.float32).astype(
            ml_dtypes.bfloat16
        ),
        np.ascontiguousarray(bdones, dtype=np.float32).astype(ml_dtypes.bfloat16),
    )


def _build_nc():
    nc = bacc.Bacc(
        "TRN2", target_bir_lowering=False, debug=False, num_devices=N_CORES
    )
    x_ap = nc.dram_tensor("x", [B_CORE, D], F32R, kind="ExternalInput").ap()
    mv1_ap = nc.dram_tensor("mv1", [128, 256], F32R, kind="ExternalInput").ap()
    mv2a_ap = nc.dram_tensor("mv2a", [128, 256], BF16, kind="ExternalInput").ap()
    mv2b_ap = nc.dram_tensor("mv2b", [128, 256], BF16, kind="ExternalInput").ap()
    bd_ap = nc.dram_tensor("bdones", [128, 128], BF16, kind="ExternalInput").ap()
    out_ap = nc.dram_tensor("probs", [B_CORE, D], F32, kind="ExternalOutput").ap()

    with tile.TileContext(nc) as tc, ExitStack() as ctx:
        consts = ctx.enter_context(tc.tile_pool(name="consts", bufs=1))
        xpool = ctx.enter_context(tc.tile_pool(name="xp", bufs=2))
        s1pool = ctx.enter_context(tc.tile_pool(name="s1p", bufs=2))
        x2pool = ctx.enter_context(tc.tile_pool(name="x2p", bufs=2))
        segpool = ctx.enter_context(tc.tile_pool(name="segp", bufs=2))
        invpool = ctx.enter_context(tc.tile_pool(name="invp", bufs=2))
        tpool = ctx.enter_context(tc.tile_pool(name="tp", bufs=3))
        pfpool = ctx.enter_context(tc.tile_pool(name="pfp", bufs=2))
        pfspool = ctx.enter_context(tc.tile_pool(name="pfsp", bufs=2))
        ps1 = ctx.enter_context(tc.tile_pool(name="ps1", bufs=2, space="PSUM"))
        ps2 = ctx.enter_context(tc.tile_pool(name="ps2", bufs=2, space="PSUM"))

        # ---- warmup: keep PE busy (HAM un-throttle) + ACT table preload ----
        wsrc = consts.tile([128, 256], F32R, tag="wsrc")
        nc.vector.memset(wsrc[:].bitcast(F32), 0.0)
        wact = consts.tile([128, 16], BF16, tag="wact")
        nc.scalar.square(wact[:], wsrc[:, 0:16].bitcast(F32))
        pw = ps2.tile([128, 1024], F32, tag="g2")
        for i in range(24):
            nc.tensor.matmul(
                pw[:, (i % 4) * 256 : (i % 4 + 1) * 256],
                lhsT=wsrc[:, 0:128],
                rhs=wsrc[:],
                start=True,
                stop=True,
            )

        # ---- constants ----
        mv1_tt = consts.tile([128, 256], F32R, tag="mv1")
        nc.sync.dma_start(mv1_tt[:], mv1_ap[:])
        mv2a_tt = consts.tile([128, 256], BF16, tag="mv2a")
        nc.sync.dma_start(mv2a_tt[:], mv2a_ap[:])
        mv2b_tt = consts.tile([128, 256], BF16, tag="mv2b")
        nc.sync.dma_start(mv2b_tt[:], mv2b_ap[:])
        bd_tt = consts.tile([128, 128], BF16, tag="bd")
        nc.sync.dma_start(bd_tt[:], bd_ap[:])
        mv1_t = mv1_tt[:]
        mv2a_t = mv2a_tt[:]
        mv2b_t = mv2b_tt[:]

        all_X = [None] * N_CHUNKS

        def emit_load(k):
            X = xpool.tile([128, D], F32R, tag="X")
            all_X[k] = X
            src = (
                x_ap[k * CHUNK : (k + 1) * CHUNK, :]
                .flatten()
                .rearrange("(bh b2 q5 l) -> (b2 q5) bh l", bh=32, b2=4, q5=32, l=128)
            )
            for h in range(2):
                nc.sync.dma_start(
                    X[:, h * 2048 : (h + 1) * 2048].rearrange(
                        "p (bh l) -> p bh l", l=128
                    ),
                    src[:, h * 16 : (h + 1) * 16, :],
                )

        emit_load(0)
        emit_load(1)

        for k in range(N_CHUNKS):
            X = all_X[k]
            S1 = s1pool.tile([128, 8192], BF16, tag="S1")
            seg = segpool.tile([128, 32], BF16, tag="seg")
            invn2 = invpool.tile([128, 32], F32, tag="invn2")
            Pf = [None, None]
            PfS = [None, None]

            for g in range(NG):
                if g % 4 == 0:
                    pf_t = pfpool.tile([128, 2048], F32, tag="Pf")
                    Pf[g // 4] = pf_t
                # stage 1 group: 4 c-tiles (bh = 4g..4g+3)
                pg = ps1.tile([128, 1024], F32, tag="g1")
                for j in range(4):
                    nc.tensor.matmul(
                        pg[:, j * 256 : (j + 1) * 256],
                        lhsT=X[:, (4 * g + j) * 128 : (4 * g + j + 1) * 128],
                        rhs=mv1_t,
                        start=True,
                        stop=True,
                    )
                # evacuate psum f32 -> S1 bf16 (split across V and S queues)
                s1c = S1[:, g * 1024 : (g + 1) * 1024]
                if g in (1, 3, 5):
                    nc.scalar.copy(s1c, pg[:])
                else:
                    nc.vector.tensor_copy(s1c, pg[:])

                # norm partials once the X cols are last-used (after stage 1)
                if g % 2 == 1:
                    q = g // 2
                    x2 = x2pool.tile([128, 1024], BF16, tag="x2")
                    nc.scalar.square(
                        x2[:], X[:, q * 1024 : (q + 1) * 1024].bitcast(F32)
                    )
                    with nc.allow_low_precision(reason="norm partials, 2e-2 tol"):
                        nc.vector.tensor_reduce(
                            seg[:, q * 8 : (q + 1) * 8],
                            x2[:].rearrange("p (bh l) -> p bh l", l=128),
                            axis=mybir.AxisListType.X,
                            op=AluOpType.add,
                        )

                # stage 2 group (needs this group's S1 only)
                pg2 = ps2.tile([128, 1024], F32, tag="g2")
                for j in range(4):
                    base = g * 1024 + j * 256
                    nc.tensor.matmul(
                        pg2[:, j * 256 : (j + 1) * 256],
                        lhsT=S1[:, base : base + 128],
                        rhs=mv2a_t,
                        start=True,
                        stop=False,
                    )
                    nc.tensor.matmul(
                        pg2[:, j * 256 : (j + 1) * 256],
                        lhsT=S1[:, base + 128 : base + 256],
                        rhs=mv2b_t,
                        start=False,
                        stop=True,
                    )
                # |amp|^2: square on ScalarE (psum->bf16), pair-add on VectorE
                T = tpool.tile([128, 1024], BF16, tag="T")
                nc.scalar.square(T[:], pg2[:])
                T4 = T[:].rearrange("p (j r c) -> p j r c", j=4, r=2)
                nc.vector.tensor_tensor(
                    Pf[g // 4][:, (g % 4) * 512 : (g % 4 + 1) * 512].rearrange(
                        "p (j c) -> p j c", c=128
                    ),
                    T4[:, :, 0],
                    T4[:, :, 1],
                    op=AluOpType.add,
                )

            # norm: sum over q5 within b2 blocks (PE), then reciprocal
            psv = ps1.tile([128, 1024], F32, tag="g1")
            nc.tensor.matmul(
                psv[:, 0:32], lhsT=bd_tt[:], rhs=seg[:], start=True, stop=True
            )
            nc.vector.reciprocal(invn2[:], psv[:, 0:32])

            if k + 2 < N_CHUNKS:
                emit_load(k + 2)

            # final scale (GpSimd) + store, per half
            oflat = (
                out_ap[k * CHUNK : (k + 1) * CHUNK, :]
                .flatten()
                .rearrange("(bh b2 q5 l) -> (b2 q5) bh l", bh=32, b2=4, q5=32, l=128)
            )
            for h in range(2):
                pfs_t = pfspool.tile([128, 2048], F32, tag="PfS")
                PfS[h] = pfs_t
                nc.gpsimd.tensor_tensor(
                    PfS[h][:].rearrange("p (bh l) -> p bh l", l=128),
                    Pf[h][:].rearrange("p (bh l) -> p bh l", l=128),
                    invn2[:, h * 16 : (h + 1) * 16]
                    .unsqueeze(2)
                    .broadcast_to([128, 16, 128]),
                    op=AluOpType.mult,
                )
                nc.sync.dma_start(
                    oflat[:, h * 16 : (h + 1) * 16, :],
                    PfS[h][:].rearrange("p (bh l) -> p bh l", l=128),
                )

    nc.compile()
    return nc


_NC_CACHE = {}


def _get_nc():
    if "nc" not in _NC_CACHE:
        _NC_CACHE["nc"] = _build_nc()
    return _NC_CACHE["nc"]


def kernel(inputs, thetas, phis, lams, _trace=False, _trace_kwargs=None):
    inputs = np.ascontiguousarray(np.asarray(inputs), dtype=np.float32)
    mv1, mv2a, mv2b, bdones = _gate_consts(thetas, phis, lams)

    nc = _get_nc()
    in_maps = [
        {
            "x": inputs[k * B_CORE : (k + 1) * B_CORE],
            "mv1": mv1,
            "mv2a": mv2a,
            "mv2b": mv2b,
            "bdones": bdones,
        }
        for k in range(N_CORES)
    ]
    res = run_bass_kernel_spmd(
        nc, in_maps, list(range(N_CORES)), trace=_trace, **(_trace_kwargs or {})
    )
    out = np.concatenate([res.results[k]["probs"] for k in range(N_CORES)], axis=0)
    if _trace:
        kernel.last_result = res
    return out


# revision 10
# speedup vs baseline: 1.1979x; 1.0711x over previous
"""Trainium2 Bass kernel for the DifferentiableQuantumCircuit problem.

Math: output = |U x / ||x|| |^2 with U = kron of 12 single-qubit U3 gates
applied twice (2 layers). Gates on different qubits commute, so the two
layers fuse into ONE kron-product unitary with per-qubit gates
G_q = U3_layer2(q) @ U3_layer1(q).

State index split: i = q5 * 128 + l7, with q5 = qubits 0-4 (5 MSBs) and
l7 = qubits 5-11 (7 LSBs, contiguous in memory -> 512B DMA bursts).
U_total = M5a (x) M7b with M5a = kron(G_0..G_4) [32x32] acting on q5 and
M7b = kron(G_5..G_11) [128x128] acting on l7.

Per-core pipeline (512 samples/core, 4 chunks of 128 samples b=(bh,b2),
bh in [0,32), b2 in [0,4)); per chunk, 8 groups of 4 bh:
  stage 1 (PE, f32r): stationary = X c-tile (fixed bh), moving =
    [Re(G5^T)|Im(G5^T)] with G5 = I4 (x) M5a -> psum[l7, (re/im,(b2,q5))]
    (applies the 5-qubit gate group AND transposes l7 onto partitions)
  evac (V/S split): psum f32 -> SBUF bf16 S1 tiles
  stage 2 (PE, bf16): stationary = S1 re/im slices, moving =
    [Re(M7b^T)|Im(M7b^T)] / [-Im|Re] accumulating -> psum[(b2,q5'), (re/im, l7')]
  squares (S/V split): psum f32 -> T bf16; pair add (G/V split) -> P bf16
  norm (early, off critical path): x^2 (ScalarE) -> per-bh l7-reduce
    (VectorE) -> block-diag-ones matmul (PE, sums over q5) -> reciprocal
    (VectorE) = 1/||x||^2
  final scale (GpSimdE): P * invnorm2 broadcast -> PfS f32 -> DMA store

Engine budget per chunk (~14us each): V = reduces + 5 evacs + 2 sqs +
2 adds + recip; S = x^2 + 3 evacs + 6 sqs; G = 6 adds + 2 half scales;
PE = 97 matmuls. Emission order is engine-FIFO-aware (queues execute in
program order; anything emitted early that waits late blocks the queue).
"""

from contextlib import ExitStack

import numpy as np
import ml_dtypes

import concourse.tile as tile
from concourse import bacc, mybir
from concourse.alu_op_type import AluOpType
from concourse.bass_utils import run_bass_kernel_spmd

F32 = mybir.dt.float32
F32R = mybir.dt.float32r
BF16 = mybir.dt.bfloat16

NUM_QUBITS = 12
D = 4096
B = 4096
N_CORES = 8
B_CORE = B // N_CORES  # 512
CHUNK = 128
N_CHUNKS = B_CORE // CHUNK  # 4
NG = 8  # groups per chunk (4 bh each)

EVAC_V = (0, 1, 2, 4, 5, 6, 7)  # stage-1 evacuation on VectorE; rest ScalarE
ADD_V = (6, 7)  # pair-adds on VectorE; rest GpSimdE


def _u3(theta, phi, lam):
    c = np.cos(theta / 2.0)
    s = np.sin(theta / 2.0)
    return np.array(
        [
            [c, -np.exp(1j * lam) * s],
            [np.exp(1j * phi) * s, np.exp(1j * (phi + lam)) * c],
        ],
        dtype=np.complex128,
    )


def _gate_consts(thetas, phis, lams):
    """Constant moving-operand matrices for both PE stages + bdones."""
    thetas = np.asarray(thetas, dtype=np.float64)
    phis = np.asarray(phis, dtype=np.float64)
    lams = np.asarray(lams, dtype=np.float64)
    gates = []
    for q in range(NUM_QUBITS):
        g1 = _u3(thetas[0, q], phis[0, q], lams[0, q])
        g2 = _u3(thetas[1, q], phis[1, q], lams[1, q])
        gates.append(g2 @ g1)  # layer 1 applied first, then layer 2

    m5a = gates[0]
    for q in range(1, 5):
        m5a = np.kron(m5a, gates[q])  # [32,32], acts on q5 (bits 0-4)
    m7b = gates[5]
    for q in range(6, 12):
        m7b = np.kron(m7b, gates[q])  # [128,128], acts on l7 (bits 5-11)

    g5 = np.kron(np.eye(4), m5a)  # [128,128] block-diag over (b2, q5)

    mv1 = np.concatenate([g5.T.real, g5.T.imag], axis=1)  # [128,256]
    mv2a = np.concatenate([m7b.T.real, m7b.T.imag], axis=1)
    mv2b = np.concatenate([-m7b.T.imag, m7b.T.real], axis=1)
    bdones = np.kron(np.eye(4), np.ones((32, 32)))  # sums over q5 per b2
    bf = ml_dtypes.bfloat16
    return (
        np.ascontiguousarray(mv1, dtype=np.float32),
        np.ascontiguousarray(mv2a, dtype=np.float32).astype(bf),
        np.ascontiguousarray(mv2b, dtype=np.float32).astype(bf),
        np.ascontiguousarray(bdones, dtype=np.float32).astype(bf),
    )


def _build_nc():
    nc = bacc.Bacc(
        "TRN2", target_bir_lowering=False, debug=False, num_devices=N_CORES
    )
    x_ap = nc.dram_tensor("x", [B_CORE, D], F32R, kind="ExternalInput").ap()
    mv1_ap = nc.dram_tensor("mv1", [128, 256], F32R, kind="ExternalInput").ap()
    mv2a_ap = nc.dram_tensor("mv2a", [128, 256], BF16, kind="ExternalInput").ap()
    mv2b_ap = nc.dram_tensor("mv2b", [128, 256], BF16, kind="ExternalInput").ap()
    bd_ap = nc.dram_tensor("bdones", [128, 128], BF16, kind="ExternalInput").ap()
    out_ap = nc.dram_tensor("probs", [B_CORE, D], F32, kind="ExternalOutput").ap()

    with tile.TileContext(nc) as tc, ExitStack() as ctx:
        consts = ctx.enter_context(tc.tile_pool(name="consts", bufs=1))
        xpool = ctx.enter_context(tc.tile_pool(name="xp", bufs=2))
        s1pool = ctx.enter_context(tc.tile_pool(name="s1p", bufs=2))
        x2pool = ctx.enter_context(tc.tile_pool(name="x2p", bufs=2))
        segpool = ctx.enter_context(tc.tile_pool(name="segp", bufs=2))
        invpool = ctx.enter_context(tc.tile_pool(name="invp", bufs=2))
        tpool = ctx.enter_context(tc.tile_pool(name="tp", bufs=4))
        pfpool = ctx.enter_context(tc.tile_pool(name="pfp", bufs=2))
        pfspool = ctx.enter_context(tc.tile_pool(name="pfsp", bufs=2))
        ps1 = ctx.enter_context(tc.tile_pool(name="ps1", bufs=2, space="PSUM"))
        ps2 = ctx.enter_context(tc.tile_pool(name="ps2", bufs=2, space="PSUM"))

        # ---- warmup: keep PE busy (HAM un-throttle) + ACT table preload ----
        wsrc = consts.tile([128, 256], F32R, tag="wsrc")
        nc.vector.memset(wsrc[:].bitcast(F32), 0.0)
        wact = consts.tile([128, 16], BF16, tag="wact")
        nc.scalar.square(wact[:], wsrc[:, 0:16].bitcast(F32))
        pw = ps2.tile([128, 1024], F32, tag="g2")
        for i in range(24):
            nc.tensor.matmul(
                pw[:, (i % 4) * 256 : (i % 4 + 1) * 256],
                lhsT=wsrc[:, 0:128],
                rhs=wsrc[:],
                start=True,
                stop=True,
            )

        # ---- constants ----
        mv1_tt = consts.tile([128, 256], F32R, tag="mv1")
        nc.sync.dma_start(mv1_tt[:], mv1_ap[:])
        mv2a_tt = consts.tile([128, 256], BF16, tag="mv2a")
        nc.sync.dma_start(mv2a_tt[:], mv2a_ap[:])
        mv2b_tt = consts.tile([128, 256], BF16, tag="mv2b")
        nc.sync.dma_start(mv2b_tt[:], mv2b_ap[:])
        bd_tt = consts.tile([128, 128], BF16, tag="bd")
        nc.sync.dma_start(bd_tt[:], bd_ap[:])
        mv1_t = mv1_tt[:]
        mv2a_t = mv2a_tt[:]
        mv2b_t = mv2b_tt[:]

        all_X = [None] * N_CHUNKS

        def emit_load(k):
            X = xpool.tile([128, D], F32R, tag="X")
            all_X[k] = X
            src = (
                x_ap[k * CHUNK : (k + 1) * CHUNK, :]
                .flatten()
                .rearrange("(bh b2 q5 l) -> (b2 q5) bh l", bh=32, b2=4, q5=32, l=128)
            )
            for h in range(2):
                nc.sync.dma_start(
                    X[:, h * 2048 : (h + 1) * 2048].rearrange(
                        "p (bh l) -> p bh l", l=128
                    ),
                    src[:, h * 16 : (h + 1) * 16, :],
                )

        emit_load(0)
        emit_load(1)

        for k in range(N_CHUNKS):
            X = all_X[k]
            S1 = s1pool.tile([128, 8192], BF16, tag="S1")
            seg = segpool.tile([128, 32], BF16, tag="seg")
            invn2 = invpool.tile([128, 32], F32, tag="invn2")
            Pf = [None, None]

            def norm_piece(q):
                # x^2 (S) -> per-bh reduce over l7 (V); bf16 partials
                x2 = x2pool.tile([128, 1024], BF16, tag="x2")
                nc.scalar.square(x2[:], X[:, q * 1024 : (q + 1) * 1024].bitcast(F32))
                with nc.allow_low_precision(reason="norm partials, 2e-2 tol"):
                    nc.vector.tensor_reduce(
                        seg[:, q * 8 : (q + 1) * 8],
                        x2[:].rearrange("p (bh l) -> p bh l", l=128),
                        axis=mybir.AxisListType.X,
                        op=AluOpType.add,
                    )

            norm_piece(0)
            norm_piece(1)

            for g in range(NG):
                if g % 4 == 0:
                    pf_t = pfpool.tile([128, 2048], BF16, tag="Pf")
                    Pf[g // 4] = pf_t
                # stage 1 group: 4 c-tiles (bh = 4g..4g+3)
                pg = ps1.tile([128, 1024], F32, tag="g1")
                for j in range(4):
                    nc.tensor.matmul(
                        pg[:, j * 256 : (j + 1) * 256],
                        lhsT=X[:, (4 * g + j) * 128 : (4 * g + j + 1) * 128],
                        rhs=mv1_t,
                        start=True,
                        stop=True,
                    )
                # evacuate psum f32 -> S1 bf16
                s1c = S1[:, g * 1024 : (g + 1) * 1024]
                if g in EVAC_V:
                    nc.vector.tensor_copy(s1c, pg[:])
                else:
                    nc.scalar.copy(s1c, pg[:])

                if g == 1:
                    # second-half norm pieces: land after half-1 DMA
                    norm_piece(2)
                    norm_piece(3)

                # stage 2 group (needs this group's S1 only)
                pg2 = ps2.tile([128, 1024], F32, tag="g2")
                for j in range(4):
                    base = g * 1024 + j * 256
                    nc.tensor.matmul(
                        pg2[:, j * 256 : (j + 1) * 256],
                        lhsT=S1[:, base : base + 128],
                        rhs=mv2a_t,
                        start=True,
                        stop=False,
                    )
                    nc.tensor.matmul(
                        pg2[:, j * 256 : (j + 1) * 256],
                        lhsT=S1[:, base + 128 : base + 256],
                        rhs=mv2b_t,
                        start=False,
                        stop=True,
                    )

                if g == 2:
                    # norm tail: sum over q5 (PE) + reciprocal (V); must be
                    # emitted before the g==3 scale that reads invn2
                    psv = ps1.tile([128, 1024], F32, tag="g1")
                    nc.tensor.matmul(
                        psv[:, 0:32], lhsT=bd_tt[:], rhs=seg[:], start=True, stop=True
                    )
                    nc.vector.reciprocal(invn2[:], psv[:, 0:32])

                # |amp|^2: square psum->bf16 T, pair-add -> P bf16
                T = tpool.tile([128, 1024], BF16, tag="T")
                nc.scalar.square(T[:], pg2[:])
                T4 = T[:].rearrange("p (j r c) -> p j r c", j=4, r=2)
                add_eng = nc.vector if g in ADD_V else nc.gpsimd
                add_eng.tensor_tensor(
                    Pf[g // 4][:, (g % 4) * 512 : (g % 4 + 1) * 512].rearrange(
                        "p (j c) -> p j c", c=128
                    ),
                    T4[:, :, 0],
                    T4[:, :, 1],
                    op=AluOpType.add,
                )

                if g % 4 == 3:
                    # half complete: scale by 1/||x||^2 (G) and store
                    h = g // 4
                    pfs_t = pfspool.tile([128, 2048], F32, tag="PfS")
                    nc.gpsimd.tensor_tensor(
                        pfs_t[:].rearrange("p (bh l) -> p bh l", l=128),
                        Pf[h][:].rearrange("p (bh l) -> p bh l", l=128),
                        invn2[:, h * 16 : (h + 1) * 16]
                        .unsqueeze(2)
                        .broadcast_to([128, 16, 128]),
                        op=AluOpType.mult,
                    )
                    oflat = (
                        out_ap[k * CHUNK : (k + 1) * CHUNK, :]
                        .flatten()
                        .rearrange(
                            "(bh b2 q5 l) -> (b2 q5) bh l", bh=32, b2=4, q5=32, l=128
                        )
                    )
                    nc.sync.dma_start(
                        oflat[:, h * 16 : (h + 1) * 16, :],
                        pfs_t[:].rearrange("p (bh l) -> p bh l", l=128),
                    )
                    if h == 1 and k + 2 < N_CHUNKS:
                        emit_load(k + 2)

    nc.compile()
    return nc


_NC_CACHE = {}


def _get_nc():
    if "nc" not in _NC_CACHE:
        _NC_CACHE["nc"] = _build_nc()
    return _NC_CACHE["nc"]


def kernel(inputs, thetas, phis, lams, _trace=False, _trace_kwargs=None):
    inputs = np.ascontiguousarray(np.asarray(inputs), dtype=np.float32)
    mv1, mv2a, mv2b, bdones = _gate_consts(thetas, phis, lams)

    nc = _get_nc()
    in_maps = [
        {
            "x": inputs[k * B_CORE : (k + 1) * B_CORE],
            "mv1": mv1,
            "mv2a": mv2a,
            "mv2b": mv2b,
            "bdones": bdones,
        }
        for k in range(N_CORES)
    ]
    res = run_bass_kernel_spmd(
        nc, in_maps, list(range(N_CORES)), trace=_trace, **(_trace_kwargs or {})
    )
    out = np.concatenate([res.results[k]["probs"] for k in range(N_CORES)], axis=0)
    if _trace:
        kernel.last_result = res
    return out


# revision 13
# speedup vs baseline: 1.3955x; 1.1650x over previous
"""Trainium2 Bass kernel for the DifferentiableQuantumCircuit problem.

Math: output = |U x / ||x|| |^2 with U = kron of 12 single-qubit U3 gates
applied twice (2 layers). Gates on different qubits commute, so the two
layers fuse into ONE kron-product unitary with per-qubit gates
G_q = U3_layer2(q) @ U3_layer1(q).

State index split: i = q5 * 128 + l7, with q5 = qubits 0-4 (5 MSBs) and
l7 = qubits 5-11 (7 LSBs, contiguous in memory -> 512B DMA bursts).
U_total = M5a (x) M7b with M5a = kron(G_0..G_4) [32x32] acting on q5 and
M7b = kron(G_5..G_11) [128x128] acting on l7.

Per-core dataflow (512 samples/core, 4 chunks of 128 samples b=(bh,b2),
bh in [0,32), b2 in [0,4)); per chunk, 8 groups of 4 bh:
  stage 1 (PE, f32r): stationary = X c-tile (fixed bh), moving =
    [Re(G5^T)|Im(G5^T)] with G5 = I4 (x) M5a -> psum[l7, (re/im,(b2,q5))]
    (applies the 5-qubit gate group AND transposes l7 onto partitions)
  evac (V/S): psum f32 -> SBUF bf16 S1 tiles
  stage 2 (PE, bf16): stationary = S1 re/im slices, moving =
    [Re(M7b^T)|Im(M7b^T)] / [-Im|Re] accumulating -> psum[(b2,q5'), (re/im, l7')]
  squares (S): psum f32 -> T bf16; pair add (V/G) -> Pf bf16
  norm (off critical path): x^2 (S) -> per-bh l7-reduce (V) ->
    block-diag-ones matmul (PE) -> reciprocal (V) = 1/||x||^2
  final scale (G): Pf * invnorm2 broadcast -> PfS f32 -> DMA store

SOFTWARE PIPELINE: engine queues are FIFO in emission order, so the
main loop k interleaves stage-2 of chunk k with stage-1 of chunk k+1.
Every cross-engine dependency then has ~a full chunk of slack instead
of serializing a per-group latency chain. Norm chain for chunk k+1 is
also emitted inside loop k, placed where its inputs have landed.
"""

from contextlib import ExitStack

import numpy as np
import ml_dtypes

import concourse.tile as tile
from concourse import bacc, mybir
from concourse.alu_op_type import AluOpType
from concourse.bass_utils import run_bass_kernel_spmd

F32 = mybir.dt.float32
F32R = mybir.dt.float32r
BF16 = mybir.dt.bfloat16

NUM_QUBITS = 12
D = 4096
B = 4096
N_CORES = 8
B_CORE = B // N_CORES  # 512
CHUNK = 128
N_CHUNKS = B_CORE // CHUNK  # 4
NG = 8  # groups per chunk (4 bh each)

EVAC_V = (0, 1, 2, 4, 5, 6, 7)  # stage-1 evacuation on VectorE; rest ScalarE
ADD_V = (0, 1)  # pair-adds on VectorE (early groups fill V's loop start)


def _u3(theta, phi, lam):
    c = np.cos(theta / 2.0)
    s = np.sin(theta / 2.0)
    return np.array(
        [
            [c, -np.exp(1j * lam) * s],
            [np.exp(1j * phi) * s, np.exp(1j * (phi + lam)) * c],
        ],
        dtype=np.complex128,
    )


def _gate_consts(thetas, phis, lams):
    """Constant moving-operand matrices for both PE stages + bdones."""
    thetas = np.asarray(thetas, dtype=np.float64)
    phis = np.asarray(phis, dtype=np.float64)
    lams = np.asarray(lams, dtype=np.float64)
    gates = []
    for q in range(NUM_QUBITS):
        g1 = _u3(thetas[0, q], phis[0, q], lams[0, q])
        g2 = _u3(thetas[1, q], phis[1, q], lams[1, q])
        gates.append(g2 @ g1)  # layer 1 applied first, then layer 2

    m5a = gates[0]
    for q in range(1, 5):
        m5a = np.kron(m5a, gates[q])  # [32,32], acts on q5 (bits 0-4)
    m7b = gates[5]
    for q in range(6, 12):
        m7b = np.kron(m7b, gates[q])  # [128,128], acts on l7 (bits 5-11)

    g5 = np.kron(np.eye(4), m5a)  # [128,128] block-diag over (b2, q5)

    mv1 = np.concatenate([g5.T.real, g5.T.imag], axis=1)  # [128,256]
    mv2a = np.concatenate([m7b.T.real, m7b.T.imag], axis=1)
    mv2b = np.concatenate([-m7b.T.imag, m7b.T.real], axis=1)
    bdones = np.kron(np.eye(4), np.ones((32, 32)))  # sums over q5 per b2
    bf = ml_dtypes.bfloat16
    return (
        np.ascontiguousarray(mv1, dtype=np.float32),
        np.ascontiguousarray(mv2a, dtype=np.float32).astype(bf),
        np.ascontiguousarray(mv2b, dtype=np.float32).astype(bf),
        np.ascontiguousarray(bdones, dtype=np.float32).astype(bf),
    )


def _build_nc():
    nc = bacc.Bacc(
        "TRN2", target_bir_lowering=False, debug=False, num_devices=N_CORES
    )
    x_ap = nc.dram_tensor("x", [B_CORE, D], F32R, kind="ExternalInput").ap()
    mv1_ap = nc.dram_tensor("mv1", [128, 256], F32R, kind="ExternalInput").ap()
    mv2a_ap = nc.dram_tensor("mv2a", [128, 256], BF16, kind="ExternalInput").ap()
    mv2b_ap = nc.dram_tensor("mv2b", [128, 256], BF16, kind="ExternalInput").ap()
    bd_ap = nc.dram_tensor("bdones", [128, 128], BF16, kind="ExternalInput").ap()
    out_ap = nc.dram_tensor("probs", [B_CORE, D], F32, kind="ExternalOutput").ap()

    with tile.TileContext(nc) as tc, ExitStack() as ctx:
        consts = ctx.enter_context(tc.tile_pool(name="consts", bufs=1))
        xpool = ctx.enter_context(tc.tile_pool(name="xp", bufs=2))
        s1pool = ctx.enter_context(tc.tile_pool(name="s1p", bufs=2))
        x2pool = ctx.enter_context(tc.tile_pool(name="x2p", bufs=2))
        segpool = ctx.enter_context(tc.tile_pool(name="segp", bufs=2))
        invpool = ctx.enter_context(tc.tile_pool(name="invp", bufs=2))
        tpool = ctx.enter_context(tc.tile_pool(name="tp", bufs=4))
        pfpool = ctx.enter_context(tc.tile_pool(name="pfp", bufs=2))
        pfspool = ctx.enter_context(tc.tile_pool(name="pfsp", bufs=2))
        ps1 = ctx.enter_context(tc.tile_pool(name="ps1", bufs=2, space="PSUM"))
        ps2 = ctx.enter_context(tc.tile_pool(name="ps2", bufs=2, space="PSUM"))

        # ---- warmup: keep PE busy (HAM un-throttle) + ACT table preload ----
        wsrc = consts.tile([128, 256], F32R, tag="wsrc")
        nc.vector.memset(wsrc[:].bitcast(F32), 0.0)
        wact = consts.tile([128, 16], BF16, tag="wact")
        nc.scalar.square(wact[:], wsrc[:, 0:16].bitcast(F32))
        pw = ps2.tile([128, 1024], F32, tag="g2")
        for i in range(24):
            nc.tensor.matmul(
                pw[:, (i % 4) * 256 : (i % 4 + 1) * 256],
                lhsT=wsrc[:, 0:128],
                rhs=wsrc[:],
                start=True,
                stop=True,
            )

        # ---- constants ----
        mv1_tt = consts.tile([128, 256], F32R, tag="mv1")
        nc.sync.dma_start(mv1_tt[:], mv1_ap[:])
        mv2a_tt = consts.tile([128, 256], BF16, tag="mv2a")
        nc.sync.dma_start(mv2a_tt[:], mv2a_ap[:])
        mv2b_tt = consts.tile([128, 256], BF16, tag="mv2b")
        nc.sync.dma_start(mv2b_tt[:], mv2b_ap[:])
        bd_tt = consts.tile([128, 128], BF16, tag="bd")
        nc.sync.dma_start(bd_tt[:], bd_ap[:])
        mv1_t = mv1_tt[:]
        mv2a_t = mv2a_tt[:]
        mv2b_t = mv2b_tt[:]

        all_X = [None] * N_CHUNKS
        all_S1 = [None] * N_CHUNKS
        all_seg = [None] * N_CHUNKS
        all_inv = [None] * N_CHUNKS
        all_Pf = [[None, None] for _ in range(N_CHUNKS)]

        def emit_load(k, pieces=2):
            X = xpool.tile([128, D], F32R, tag="X")
            all_X[k] = X
            src = (
                x_ap[k * CHUNK : (k + 1) * CHUNK, :]
                .flatten()
                .rearrange("(bh b2 q5 l) -> (b2 q5) bh l", bh=32, b2=4, q5=32, l=128)
            )
            w = 32 // pieces
            for h in range(pieces):
                nc.sync.dma_start(
                    X[:, h * w * 128 : (h + 1) * w * 128].rearrange(
                        "p (bh l) -> p bh l", l=128
                    ),
                    src[:, h * w : (h + 1) * w, :],
                )

        def s1_group(k, g):
            """Stage-1 matmuls for 4 bh + psum->S1 bf16 evacuation."""
            if g == 0:
                S1 = s1pool.tile([128, 8192], BF16, tag="S1")
                all_S1[k] = S1
            X = all_X[k]
            S1 = all_S1[k]
            pg = ps1.tile([128, 1024], F32, tag="g1")
            for j in range(4):
                nc.tensor.matmul(
                    pg[:, j * 256 : (j + 1) * 256],
                    lhsT=X[:, (4 * g + j) * 128 : (4 * g + j + 1) * 128],
                    rhs=mv1_t,
                    start=True,
                    stop=True,
                )
            s1c = S1[:, g * 1024 : (g + 1) * 1024]
            if g in EVAC_V:
                nc.vector.tensor_copy(s1c, pg[:])
            else:
                nc.scalar.copy(s1c, pg[:])

        def norm_front(k, q):
            """x^2 (S) + per-bh reduce over l7 (V) for quarter q."""
            if q == 0:
                seg = segpool.tile([128, 32], BF16, tag="seg")
                all_seg[k] = seg
            X = all_X[k]
            seg = all_seg[k]
            x2 = x2pool.tile([128, 1024], BF16, tag="x2")
            nc.scalar.square(x2[:], X[:, q * 1024 : (q + 1) * 1024].bitcast(F32))
            with nc.allow_low_precision(reason="norm partials, 2e-2 tol"):
                nc.vector.tensor_reduce(
                    seg[:, q * 8 : (q + 1) * 8],
                    x2[:].rearrange("p (bh l) -> p bh l", l=128),
                    axis=mybir.AxisListType.X,
                    op=AluOpType.add,
                )

        def norm_tail(k):
            """Sum over q5 (PE block-diag ones) + reciprocal -> 1/||x||^2."""
            invn2 = invpool.tile([128, 32], F32, tag="invn2")
            all_inv[k] = invn2
            psv = ps1.tile([128, 1024], F32, tag="g1")
            nc.tensor.matmul(
                psv[:, 0:32], lhsT=bd_tt[:], rhs=all_seg[k][:], start=True, stop=True
            )
            nc.vector.reciprocal(invn2[:], psv[:, 0:32])

        def s2_group(k, g):
            """Stage-2 matmuls + squares + pair-add for 4 bh."""
            S1 = all_S1[k]
            if g % 4 == 0:
                pf_t = pfpool.tile([128, 2048], BF16, tag="Pf")
                all_Pf[k][g // 4] = pf_t
            pg2 = ps2.tile([128, 1024], F32, tag="g2")
            for j in range(4):
                base = g * 1024 + j * 256
                nc.tensor.matmul(
                    pg2[:, j * 256 : (j + 1) * 256],
                    lhsT=S1[:, base : base + 128],
                    rhs=mv2a_t,
                    start=True,
                    stop=False,
                )
                nc.tensor.matmul(
                    pg2[:, j * 256 : (j + 1) * 256],
                    lhsT=S1[:, base + 128 : base + 256],
                    rhs=mv2b_t,
                    start=False,
                    stop=True,
                )
            T = tpool.tile([128, 1024], BF16, tag="T")
            nc.scalar.square(T[:], pg2[:])
            T4 = T[:].rearrange("p (j r c) -> p j r c", j=4, r=2)
            # last chunk: V is idle during drain, G would straggle
            add_eng = (
                nc.vector if (g in ADD_V or k == N_CHUNKS - 1) else nc.gpsimd
            )
            add_eng.tensor_tensor(
                all_Pf[k][g // 4][:, (g % 4) * 512 : (g % 4 + 1) * 512].rearrange(
                    "p (j c) -> p j c", c=128
                ),
                T4[:, :, 0],
                T4[:, :, 1],
                op=AluOpType.add,
            )

        def scale_store(k, h):
            """1/||x||^2 scale (GpSimd; split with V on the drain chunk),
            then DMA the half back out."""
            pfs_t = pfspool.tile([128, 2048], F32, tag="PfS")
            eng = nc.vector if (k == N_CHUNKS - 1 and h == 1) else nc.gpsimd
            eng.tensor_tensor(
                pfs_t[:].rearrange("p (bh l) -> p bh l", l=128),
                all_Pf[k][h][:].rearrange("p (bh l) -> p bh l", l=128),
                all_inv[k][:, h * 16 : (h + 1) * 16]
                .unsqueeze(2)
                .broadcast_to([128, 16, 128]),
                op=AluOpType.mult,
            )
            oflat = (
                out_ap[k * CHUNK : (k + 1) * CHUNK, :]
                .flatten()
                .rearrange("(bh b2 q5 l) -> (b2 q5) bh l", bh=32, b2=4, q5=32, l=128)
            )
            nc.sync.dma_start(
                oflat[:, h * 16 : (h + 1) * 16, :],
                pfs_t[:].rearrange("p (bh l) -> p bh l", l=128),
            )

        # ---- flat software pipeline over (chunk, group) steps ----
        # s1(t) runs OFF steps ahead of s2(t-OFF); every cross-engine dep
        # gets ~OFF group-times of slack without doubling fill/drain.
        OFF = 6
        emit_load(0, pieces=4)
        emit_load(1)
        for t in range(N_CHUNKS * NG + OFF):
            if t >= OFF:
                k2, g2 = divmod(t - OFF, NG)
                s2_group(k2, g2)
            if t < N_CHUNKS * NG:
                k1, g1 = divmod(t, NG)
                s1_group(k1, g1)
                if g1 == 2:
                    norm_front(k1, 0)
                    norm_front(k1, 1)
                elif g1 == 4:
                    norm_front(k1, 2)
                    norm_front(k1, 3)
                elif g1 == 6:
                    norm_tail(k1)
                elif g1 == 7 and k1 + 2 < N_CHUNKS:
                    emit_load(k1 + 2)
            if t >= OFF:
                if g2 == 3:
                    scale_store(k2, 0)
                elif g2 == 7:
                    scale_store(k2, 1)

    nc.compile()
    return nc


_NC_CACHE = {}


def _get_nc():
    if "nc" not in _NC_CACHE:
        _NC_CACHE["nc"] = _build_nc()
    return _NC_CACHE["nc"]


def kernel(inputs, thetas, phis, lams, _trace=False, _trace_kwargs=None):
    inputs = np.ascontiguousarray(np.asarray(inputs), dtype=np.float32)
    mv1, mv2a, mv2b, bdones = _gate_consts(thetas, phis, lams)

    nc = _get_nc()
    in_maps = [
        {
            "x": inputs[k * B_CORE : (k + 1) * B_CORE],
            "mv1": mv1,
            "mv2a": mv2a,
            "mv2b": mv2b,
            "bdones": bdones,
        }
        for k in range(N_CORES)
    ]
    res = run_bass_kernel_spmd(
        nc, in_maps, list(range(N_CORES)), trace=_trace, **(_trace_kwargs or {})
    )
    out = np.concatenate([res.results[k]["probs"] for k in range(N_CORES)], axis=0)
    if _trace:
        kernel.last_result = res
    return out
